# revision 44
# baseline (speedup 1.0000x reference)
"""Trainium2 Bass kernel for AnticipatoryRestaurantGNN (TransformerConv x4 + BN + pool).

Strategy (edge-parallel, dst-sorted):
  - Sort edges by dst; partition nodes into 8 contiguous ranges with ~equal
    edge counts. Each core owns its node range and ALL edges pointing into it,
    so segment-softmax and scatter-add are core-local.
  - Per layer, each core computes q/k/v projections for its own nodes only;
    k/v (bf16) are AllGathered so every core can gather k[src], v[src] rows
    for its edge shard with one batched indirect DMA per 128-dst node group.
  - Edge-attr projection e = ea@We is never materialized per edge:
      logit = q.(k+e) = q.k + qE.ea       with qE = x @ (Wq_h We_h^T) (folded
                                          into the q projection, 64 extra cols)
      out   = sum a(v+e) = sum a v  +  B @ We_blk   with B = at^T @ (a (x) ea)
    accumulated per dst-group on the TensorEngine.
  - Softmax uses exp(logit) directly (shift-invariant; logits are O(1) here):
    accumulate sum(alpha*v), B, and sum(alpha) per dst node via host-baked
    one-hot scatter matmuls (PSUM accumulate), normalized in the node phase.
  - BatchNorm stats and the final pooled head are AllReduced (tiny).
  - Per-edge-tile work on DVE is minimized: fused multiply+accumulate dots
    (scalar_tensor_tensor with accum_out), PSUM->SBUF staging on the (idle)
    Activation engine, per-group batched DMA for gather/one-hots/edge-attrs.
"""

import math
import os
import sys

sys.path.insert(0, "/opt/trn_rl_repo")

import ml_dtypes
import numpy as np

import concourse.bacc as bacc
import concourse.bass as bass
import concourse.mybir as mybir
import concourse.tile as tile
from concourse.bass_utils import run_bass_kernel_spmd
from concourse.masks import make_identity

BF16 = ml_dtypes.bfloat16

N, E, IN_DIM, EDGE_DIM, HID, L, HEADS, G = 50000, 500000, 64, 16, 256, 4, 4, 64
C = HID // HEADS
NCORES = 8
P = 128
EPS = 1e-5
QW = HID + HEADS * EDGE_DIM  # 320: q | qE
MOV = HID + HEADS * EDGE_DIM + HEADS  # 324: [alpha*ea(64) | alpha(4) | alpha*v(256)]

F32 = mybir.dt.float32
BF = mybir.dt.bfloat16

AX = mybir.AxisListType
ALU = mybir.AluOpType
ACTF = mybir.ActivationFunctionType


def _roundup(x, m):
    return (x + m - 1) // m * m


def _tctile(tc, *a, **kw):
    t, _free = tc.tile(*a, **kw)
    return t


def plan(edge_index, batch):
    """Host-side layout planning. Returns (meta, per_core_arrays)."""
    src, dst = np.asarray(edge_index[0]), np.asarray(edge_index[1])
    batch = np.asarray(batch)

    order = np.argsort(dst, kind="stable")
    s_src = src[order].astype(np.int64)
    s_dst = dst[order].astype(np.int64)

    deg = np.bincount(dst, minlength=N)
    cum = np.concatenate([[0], np.cumsum(deg)])  # cum[n] = first edge of node n

    # node range split, balanced by edge count, at node boundaries
    ns = [0]
    for i in range(1, NCORES):
        tgt = round(E * i / NCORES)
        ns.append(int(np.searchsorted(cum, tgt, side="left")))
    ns.append(N)
    ns = np.array(ns, dtype=np.int64)
    n_own = np.diff(ns)
    n_pad = _roundup(int(n_own.max()), P)
    NG = n_pad // P
    ROWS = NCORES * n_pad
    BANKN = ROWS // 2
    assert BANKN <= 32767

    core_of = np.searchsorted(ns[1:], np.arange(N), side="right")
    padrow = core_of * n_pad + (np.arange(N) - ns[core_of])
    src_p = padrow[s_src]  # padded global row per sorted edge

    # per (core, group) edge lists (edges sorted by src for HBM locality)
    per_core_ed = []
    Tmax = np.zeros(NG, dtype=np.int64)
    for c in range(NCORES):
        e1 = cum[ns[c + 1]]
        groups = []
        for g in range(NG):
            lo_node = ns[c] + g * P
            hi_node = min(ns[c] + (g + 1) * P, ns[c + 1])
            if lo_node >= ns[c + 1]:
                eidx = np.arange(e1, e1)
            else:
                eidx = np.arange(cum[lo_node], cum[hi_node])
            eidx = eidx[np.argsort(src_p[eidx], kind="stable")]
            groups.append(eidx)
            Tmax[g] = max(Tmax[g], _roundup(len(eidx), P) // P)
        per_core_ed.append(groups)

    TOTE = int(Tmax.sum()) * P  # padded edges per core (same on all cores)

    counts = np.bincount(batch, minlength=G).astype(np.float64)

    meta = dict(ns=ns, n_pad=n_pad, NG=NG, ROWS=ROWS, BANKN=BANKN,
                Tmax=Tmax, TOTE=TOTE, order=order, counts=counts)

    cores = []
    for c in range(NCORES):
        kv_idx = np.zeros(TOTE, dtype=np.int32)
        at_all = np.zeros((TOTE, P), dtype=np.float32)
        ea_sel = np.full(TOTE, -1, dtype=np.int64)
        off = 0
        for g in range(NG):
            lo_node = ns[c] + g * P
            el = per_core_ed[c][g]
            T = int(Tmax[g])
            if T == 0:
                continue
            npad = T * P
            k = len(el)
            kvv = np.zeros(npad, dtype=np.int64)
            kvv[:k] = src_p[el]
            kv_idx[off:off + npad] = kvv.astype(np.int32)
            dr = np.full(npad, -1, dtype=np.int64)
            if k:
                dr[:k] = s_dst[el] - lo_node
            valid = dr >= 0
            atb = np.zeros((npad, P), dtype=np.float32)
            atb[np.arange(npad)[valid], dr[valid]] = 1.0
            at_all[off:off + npad] = atb
            ea_sel[off:off + k] = el
            off += npad
        assert off == TOTE

        # kv gather indices: [128, TT] column-per-tile (idx j -> [j%128, j//128])
        TT = TOTE // P
        kv_idx32 = np.ascontiguousarray(kv_idx.reshape(TT, P).T)

        # fused per-edge operand block: [at(128) | at^T(128) | ea4 slot(64)]
        acat = np.zeros((TT, P, 2 * P + HEADS * EDGE_DIM), dtype=BF16)
        for t in range(TT):
            blk = at_all[t * P:(t + 1) * P]
            acat[t, :, 0:P] = blk.astype(BF16)
            acat[t, :, P:2 * P] = blk.T.astype(BF16)

        nn = int(n_own[c])
        nodes = np.arange(ns[c], ns[c + 1])
        invcnt_t = np.zeros((P, NG), dtype=np.float32)
        maskcol_t = np.zeros((P, NG), dtype=np.float32)
        flat_inv = np.zeros(n_pad, dtype=np.float32)
        flat_inv[:nn] = 1.0 / np.maximum(counts[batch[nodes]], 1.0)
        flat_msk = np.zeros(n_pad, dtype=np.float32)
        flat_msk[:nn] = 1.0
        invcnt_t[:, :] = flat_inv.reshape(NG, P).T
        maskcol_t[:, :] = flat_msk.reshape(NG, P).T
        p_t = np.zeros((n_pad, G), dtype=np.float32)
        p_t[np.arange(nn), batch[nodes]] = 1.0
        maskrep = np.repeat(flat_msk[(NG - 1) * P:NG * P][None, :], P, axis=0)

        cores.append(dict(kv_idx32=kv_idx32, acat=acat, ea_sel=ea_sel,
                          invcnt_t=invcnt_t, maskcol_t=maskcol_t,
                          maskrep=maskrep.astype(BF16), p_t=p_t.astype(BF16),
                          nn=nn))
    return meta, cores


def build_inmaps(inputs, meta, cores):
    """Build the per-core in_maps dict for run_bass_kernel_spmd."""
    ns, n_pad, NG = meta["ns"], meta["n_pad"], meta["NG"]
    order = meta["order"]
    ea_sorted = np.asarray(inputs["edge_attr"], dtype=np.float32)[order]

    def f32(x):
        return np.ascontiguousarray(np.asarray(x, dtype=np.float32))

    def bf(x):
        return np.ascontiguousarray(np.asarray(x, dtype=np.float32).astype(BF16))

    nf = np.asarray(inputs["node_features"])
    Wbeta = np.asarray(inputs["Wbeta"])  # [L, 768, 1]
    head_W = np.asarray(inputs["head_W"])  # [512, 1]

    shared = {}
    shared["proj_W"] = bf(inputs["proj_W"])                      # [64,256]
    shared["proj_b"] = bf(inputs["proj_b"][None, :])             # [1,256]
    for l in range(L):
        Wq = np.asarray(inputs["Wq"][l], dtype=np.float64)
        We = np.asarray(inputs["We"][l], dtype=np.float64)       # [16, 256]
        bq = np.asarray(inputs["bq"][l], dtype=np.float64)
        # q projection extended with qE_h = q_h @ We_h^T (folds e into logit)
        Wq_p = np.zeros((HID, QW), dtype=np.float64)
        bq_p = np.zeros(QW, dtype=np.float64)
        Wq_p[:, :HID] = Wq
        bq_p[:HID] = bq
        for h in range(HEADS):
            We_h = We[:, h * C:(h + 1) * C]                      # [16, 64]
            Wq_p[:, HID + h * EDGE_DIM:HID + (h + 1) * EDGE_DIM] = \
                Wq[:, h * C:(h + 1) * C] @ We_h.T
            bq_p[HID + h * EDGE_DIM:HID + (h + 1) * EDGE_DIM] = \
                bq[h * C:(h + 1) * C] @ We_h.T
        shared[f"Wq{l}"] = bf(Wq_p)
        shared[f"bq{l}"] = bf(bq_p[None, :])
        shared[f"Wk{l}"] = bf(inputs["Wk"][l])
        shared[f"Wv{l}"] = bf(inputs["Wv"][l])
        shared[f"Wsk{l}"] = bf(inputs["Wskip"][l])
        shared[f"bk{l}"] = bf((inputs["bk"][l] + inputs["be"][l])[None, :])
        shared[f"bv{l}"] = bf((inputs["bv"][l] + inputs["be"][l])[None, :])
        shared[f"bsk{l}"] = bf(inputs["bskip"][l][None, :])
        # block-diagonal We for the per-group B @ We_blk edge contribution
        We_blk = np.zeros((HEADS * EDGE_DIM, HID), dtype=np.float64)
        for h in range(HEADS):
            We_blk[h * EDGE_DIM:(h + 1) * EDGE_DIM, h * C:(h + 1) * C] = \
                We[:, h * C:(h + 1) * C]
        shared[f"Web{l}"] = bf(We_blk)
        w1, w2, w3 = Wbeta[l, :HID, 0], Wbeta[l, HID:2 * HID, 0], Wbeta[l, 2 * HID:, 0]
        shared[f"wa{l}"] = bf(np.repeat((w1 + w3)[None, :], P, axis=0))   # [128,256]
        shared[f"wb{l}"] = bf(np.repeat((w2 - w3)[None, :], P, axis=0))
        shared[f"gam{l}"] = f32(inputs["bn_gamma"][l][None, :])
        shared[f"bet{l}"] = f32(inputs["bn_beta"][l][None, :])
    shared["h1r"] = bf(np.repeat(head_W[:HID, 0][None, :], P, axis=0))
    shared["h2r"] = bf(np.repeat(head_W[HID:, 0][None, :], P, axis=0))

    in_maps = []
    for c in range(NCORES):
        cc = cores[c]
        m = dict(shared)
        nn = cc["nn"]
        nft = np.zeros((IN_DIM, n_pad), dtype=np.float32)
        nft[:, :nn] = nf[ns[c]:ns[c + 1]].T
        m["nf_t"] = nft.astype(BF16)
        # fill ea4 slot of acat: ea tiled x4 heads
        acat = cc["acat"].copy()
        sel = cc["ea_sel"]
        ea_full = np.zeros((meta["TOTE"], EDGE_DIM), dtype=np.float32)
        ea_full[sel >= 0] = ea_sorted[sel[sel >= 0]]
        ea4 = np.tile(ea_full, (1, HEADS)).astype(BF16)          # [TOTE, 64]
        acat[:, :, 2 * P:] = ea4.reshape(meta["TOTE"] // P, P, HEADS * EDGE_DIM)
        m["acat"] = np.ascontiguousarray(acat.reshape(meta["TOTE"], 2 * P + HEADS * EDGE_DIM))
        m["kv_idx32"] = cc["kv_idx32"]                           # [128, TT] int32
        m["invcnt_t"] = cc["invcnt_t"]
        m["maskcol_t"] = cc["maskcol_t"]
        m["maskrep"] = cc["maskrep"]
        m["p_t"] = cc["p_t"]
        in_maps.append(m)
    return in_maps


def build(nc, meta, head_b):
    n_pad, NG, ROWS, Tmax, TOTE = (meta["n_pad"], meta["NG"], meta["ROWS"],
                                   meta["Tmax"], meta["TOTE"])
    TT = TOTE // P
    TMX = int(Tmax.max())
    AC = 2 * P + HEADS * EDGE_DIM  # 320 cols of acat

    # ---- dram parameters ----
    dp = lambda name, shape, dt: nc.declare_dram_parameter(name, list(shape), dt, isOutput=False)
    nf_t = dp("nf_t", (IN_DIM, n_pad), BF)
    acat = dp("acat", (TOTE, AC), BF)
    kv_idx32 = dp("kv_idx32", (P, TT), mybir.dt.int32)
    invcnt_t = dp("invcnt_t", (P, NG), F32)
    maskcol_t = dp("maskcol_t", (P, NG), F32)
    maskrep = dp("maskrep", (P, P), BF)
    p_t = dp("p_t", (n_pad, G), BF)
    proj_W = dp("proj_W", (IN_DIM, HID), BF)
    proj_b = dp("proj_b", (1, HID), BF)
    Wq, Wk, Wv, Wsk, bq, bk, bv, bsk, Web, wa, wb, gam, bet = ({} for _ in range(13))
    for l in range(L):
        Wq[l] = dp(f"Wq{l}", (HID, QW), BF)
        Wk[l] = dp(f"Wk{l}", (HID, HID), BF)
        Wv[l] = dp(f"Wv{l}", (HID, HID), BF)
        Wsk[l] = dp(f"Wsk{l}", (HID, HID), BF)
        bq[l] = dp(f"bq{l}", (1, QW), BF)
        bk[l] = dp(f"bk{l}", (1, HID), BF)
        bv[l] = dp(f"bv{l}", (1, HID), BF)
        bsk[l] = dp(f"bsk{l}", (1, HID), BF)
        Web[l] = dp(f"Web{l}", (HEADS * EDGE_DIM, HID), BF)
        wa[l] = dp(f"wa{l}", (P, HID), BF)
        wb[l] = dp(f"wb{l}", (P, HID), BF)
        gam[l] = dp(f"gam{l}", (1, HID), F32)
        bet[l] = dp(f"bet{l}", (1, HID), F32)
    h1r = dp("h1r", (P, HID), BF)
    h2r = dp("h2r", (P, HID), BF)
    out_ext = nc.declare_dram_parameter("out", [G, 1], F32, isOutput=True)
    KB_DUMP = os.environ.get("KB_DUMP", "") == "1"
    if KB_DUMP:
        d_den = nc.declare_dram_parameter("d_den", [P, NG * HEADS], F32, isOutput=True)
        d_xn = nc.declare_dram_parameter("d_xn", [P, L * NG * HID], F32, isOutput=True)
        d_xt = nc.declare_dram_parameter("d_xt", [P, 2 * n_pad], F32, isOutput=True)
        d_bn = nc.declare_dram_parameter("d_bn", [1, 2 * HID], F32, isOutput=True)

    cg = [list(range(NCORES))]
    SIM1 = os.environ.get("KB_SIM1", "") == "1"

    # WW stationary layout offsets: (col0, width, bias0)
    WOFF = {0: (0, QW, 0), 1: (2 * QW, HID, QW), 2: (2 * QW + 2 * HID, HID, QW + HID),
            3: (2 * QW + 4 * HID, HID, QW + 2 * HID)}
    WWC = 2 * QW + 6 * HID
    BWC = QW + 3 * HID

    with tile.TileContext(nc) as tc:
        # ---- persistent DRAM ----
        kv_slab = _tctile(tc, [n_pad, 2 * HID], BF, space="DRAM", name="kv_slab")
        q_slab = _tctile(tc, [n_pad, QW], BF, space="DRAM", name="q_slab")
        kv_full = _tctile(tc, [ROWS, 2 * HID], BF, space="DRAM", addr_space="Shared", name="kv_full")
        st_in = _tctile(tc, [2, HID], F32, space="DRAM", name="st_in")
        st_out = _tctile(tc, [2, HID], F32, space="DRAM", addr_space="Shared", name="st_out")
        pool_in = _tctile(tc, [G, 1], F32, space="DRAM", name="pool_in")
        pool_out = _tctile(tc, [G, 1], F32, space="DRAM", addr_space="Shared", name="pool_out")

        # ---- persistent SBUF ----
        from contextlib import ExitStack
        ctx = ExitStack()
        res = ctx.enter_context(tc.tile_pool(name="res", bufs=1))
        x_T = [res.tile([P, n_pad], BF, tag=f"xT{h}", name=f"xT{h}") for h in range(2)]
        xnew = res.tile([P, NG * HID], BF, tag="xnew", name="xnew")
        den_sb = res.tile([P, NG * HEADS], F32, tag="densb", name="densb")
        ident = res.tile([P, P], BF, tag="ident", name="ident")
        make_identity(nc, ident[:])
        ones_row = res.tile([1, P], BF, tag="onesr", name="onesr")
        nc.vector.memset(ones_row[:], 1.0)
        ones_col = res.tile([P, 1], BF, tag="onesc", name="onesc")
        nc.vector.memset(ones_col[:], 1.0)
        one1 = res.tile([1, 1], BF, tag="one1", name="one1")
        nc.vector.memset(one1[:], 1.0)
        wa_sb = res.tile([P, HID], BF, tag="wasb", name="wasb")
        wb_sb = res.tile([P, HID], BF, tag="wbsb", name="wbsb")
        aT_sb = res.tile([P, 2], F32, tag="aTsb", name="aTsb")   # col h = scale for half h
        cT_sb = res.tile([P, 2], F32, tag="cTsb", name="cTsb")
        WW = res.tile([P, WWC], BF, tag="WW", name="WW")
        Web_sb = res.tile([HEADS * EDGE_DIM, HID], BF, tag="Websb", name="Websb")
        kvidx_sb = res.tile([P, TT], mybir.dt.int32, tag="kvidx", name="kvidx")
        nc.sync.dma_start(out=kvidx_sb[:], in_=kv_idx32[:])
        bias_sb = res.tile([1, BWC], BF, tag="biassb", name="biassb")
        mct_sb = res.tile([P, NG], F32, tag="mctsb", name="mctsb")
        nc.sync.dma_start(out=mct_sb[:], in_=maskcol_t[:])
        ict_sb = res.tile([P, NG], F32, tag="ictsb", name="ictsb")
        nc.sync.dma_start(out=ict_sb[:], in_=invcnt_t[:])

        h1r_sb = res.tile([P, HID], BF, tag="h1rsb", name="h1rsb")
        nc.sync.dma_start(out=h1r_sb[:], in_=h1r[:])
        h2r_sb = res.tile([P, HID], BF, tag="h2rsb", name="h2rsb")
        nc.sync.dma_start(out=h2r_sb[:], in_=h2r[:])

        KB_SKIP = set(os.environ.get("KB_SKIP", "").split(","))
        for h in range(2):
            nc.vector.memset(x_T[h][:], 0.01)
        nc.vector.memset(xnew[:], 0.0)
        nc.vector.memset(den_sb[:], 1.0)
        nc.vector.memset(aT_sb[:], 1.0)
        nc.vector.memset(cT_sb[:], 0.0)

        wpool = ctx.enter_context(tc.tile_pool(name="wtmp", bufs=1))
        psum = ctx.enter_context(tc.tile_pool(name="ps", bufs=2, space="PSUM"))
        sb = ctx.enter_context(tc.tile_pool(name="sb", bufs=3))

        def load_layer_weights(l):
            for i, W in enumerate((Wq[l], Wk[l], Wv[l], Wsk[l])):
                w0, w, _ = WOFF[i]
                for h in range(2):
                    nc.sync.dma_start(out=WW[:, w0 + h * w:w0 + (h + 1) * w],
                                      in_=W[h * P:(h + 1) * P, :])
            nc.sync.dma_start(out=Web_sb[:], in_=Web[l][:])
            for i, b in enumerate((bq[l], bk[l], bv[l], bsk[l])):
                _, w, b0 = WOFF[i]
                nc.sync.dma_start(out=bias_sb[:, b0:b0 + w], in_=b[:])
            nc.sync.dma_start(out=wa_sb[:], in_=wa[l][:])
            nc.sync.dma_start(out=wb_sb[:], in_=wb[l][:])

        def proj_psum(t, widx):
            """x_T node-tile t through weight widx (0..3 -> q,k,v,skip) -> PSUM."""
            w0, w, b0 = WOFF[widx]
            ps = psum.tile([P, QW], F32, tag="mm", name="projps")
            nc.tensor.matmul(ps[:, :w], x_T[0][:, t * P:(t + 1) * P],
                             WW[:, w0:w0 + w], start=True, stop=False)
            nc.tensor.matmul(ps[:, :w], x_T[1][:, t * P:(t + 1) * P],
                             WW[:, w0 + w:w0 + 2 * w], start=False, stop=False)
            nc.tensor.matmul(ps[:, :w], ones_row[:],
                             bias_sb[:, b0:b0 + w], start=False, stop=True)
            return ps

        def qkv_phase():
            """compute q/k/v slabs from x_T, all-gather k/v."""
            if "qkv" in KB_SKIP:
                return
            with nc.named_scope("qkv"):
                for t in range(NG):
                    qp = proj_psum(t, 0)
                    qrow = sb.tile([P, QW], BF, tag="qrow", name="qrow")
                    nc.scalar.activation(qrow[:], qp[:], ACTF.Copy)
                    nc.sync.dma_start(out=q_slab[t * P:(t + 1) * P, :], in_=qrow[:])
                    kvrow = sb.tile([P, 2 * HID], BF, tag="kvrow", name="kvrow")
                    kp = proj_psum(t, 1)
                    nc.scalar.activation(kvrow[:, :HID], kp[:, :HID], ACTF.Copy)
                    vp = proj_psum(t, 2)
                    nc.scalar.activation(kvrow[:, HID:], vp[:, :HID], ACTF.Copy)
                    nc.sync.dma_start(out=kv_slab[t * P:(t + 1) * P, :], in_=kvrow[:])
            with nc.named_scope("allgather"):
                if SIM1:
                    for c in range(NCORES):
                        nc.sync.dma_start(out=kv_full[c * n_pad:(c + 1) * n_pad, :],
                                          in_=kv_slab[:])
                else:
                    nc.gpsimd.collective_compute(
                        "AllGather", ALU.bypass, replica_groups=cg,
                        ins=[kv_slab[:].opt()], outs=[kv_full[:].opt()])

        # ================= init: x0 = nf @ proj_W + proj_b =================
        nfs = wpool.tile([IN_DIM, n_pad], BF, tag="nfs", name="nfs")
        nc.sync.dma_start(out=nfs[:], in_=nf_t[:])
        pw_sb = wpool.tile([IN_DIM, HID], BF, tag="pw", name="pw")
        nc.sync.dma_start(out=pw_sb[:], in_=proj_W[:])
        pb_sb = wpool.tile([1, HID], BF, tag="pb", name="pb")
        nc.sync.dma_start(out=pb_sb[:], in_=proj_b[:])
        for t in range(NG if "x0" not in KB_SKIP else 0):
            ps = psum.tile([P, QW], F32, tag="mm", name="x0ps")
            nc.tensor.matmul(ps[:, :HID], nfs[:, t * P:(t + 1) * P], pw_sb[:], start=True, stop=False)
            nc.tensor.matmul(ps[:, :HID], ones_row[:], pb_sb[:], start=False, stop=True)
            xr0 = sb.tile([P, HID], BF, tag="xrow0", name="xrow0")
            nc.scalar.activation(xr0[:], ps[:, :HID], ACTF.Copy)
            for h in range(2):
                tp = psum.tile([P, P], BF, tag="tp", name="x0tp", bufs=1)
                nc.tensor.transpose(tp[:], xr0[:, h * P:(h + 1) * P], ident[:])
                nc.scalar.activation(x_T[h][:, t * P:(t + 1) * P], tp[:], ACTF.Copy)

        load_layer_weights(0)
        qkv_phase()

        # ================= layers =================
        for l in range(L):
            # ---- edge phase (node phase A fused into each group tail) ----
            KB_NG = int(os.environ.get("KB_NG", "9999"))
            KB_L = int(os.environ.get("KB_L", "9999"))
            SKIP_A = "A" in KB_SKIP
            _sc_edge = nc.enter_named_scope(f"edge{l}", False)[0]
            active = [g for g in range(NG)
                      if Tmax[g] > 0 and g < KB_NG and l < KB_L]
            # chunked stats accumulation (two-level sum: less f32 error growth)
            NCH = int(os.environ.get("KB_NCH", "1"))
            st_part = sb.tile([1, 5 * 2 * HID], F32, tag="stpart",
                              name="stpart", bufs=1)
            nc.vector.memset(st_part[:], 0.0)
            chunks = [list(a) for a in np.array_split(np.array(active), NCH)]
            ch_of = {}
            for ci, ch in enumerate(chunks):
                for g in ch:
                    ch_of[g] = (ci, g == ch[0], g == ch[-1])
            cur_ps = [None]
            if SKIP_A or not active:
                cur_ps[0] = psum.tile([1, 2 * HID], F32, tag="stats",
                                      name="stps", bufs=2)
                nc.tensor.matmul(cur_ps[0][:, :HID], ones_col[:], xnew[:, 0:HID],
                                 start=True, stop=True)
            toff = 0  # global edge-tile counter
            for g in range(NG):
                T = int(Tmax[g])
                if g not in active:
                    if T:
                        toff += T
                    nc.vector.memset(xnew[:, g * HID:(g + 1) * HID], 0.0)
                    nc.vector.memset(den_sb[:, g * HEADS:(g + 1) * HEADS], 0.0)
                    continue
                qg_sb = sb.tile([P, QW], BF, tag="qgrp", name="qgrp", bufs=2)
                nc.sync.dma_start(out=qg_sb[:], in_=q_slab[g * P:(g + 1) * P, :])
                KVB_BUFS = 1 if KB_DUMP else 2
                kvb = sb.tile([P, TMX, 2 * HID], BF, tag="kvb", name="kvb",
                              bufs=KVB_BUFS)
                for ti in range(T):
                    nc.gpsimd.indirect_dma_start(
                        out=kvb[:, ti, :], out_offset=None, in_=kv_full[:],
                        in_offset=bass.IndirectOffsetOnAxis(
                            ap=kvidx_sb[:, toff + ti:toff + ti + 1], axis=0))
                aeb = sb.tile([P, TMX, AC], BF, tag="aeb", name="aeb", bufs=2)
                nc.sync.dma_start(
                    out=aeb[:, :T, :],
                    in_=acat[toff * P:(toff + T) * P, :].rearrange(
                        "(t p) c -> p t c", p=P))
                ps_all = psum.tile([P, MOV], F32, tag="acc", name="psall")
                for ti in range(T):
                    qps = psum.tile([P, QW], F32, tag="mm2", name="qps", bufs=1)
                    nc.tensor.matmul(qps[:], aeb[:, ti, P:2 * P], qg_sb[:],
                                     start=True, stop=True)
                    qall = sb.tile([P, QW], BF, tag="qall", name="qall")
                    nc.scalar.activation(qall[:], qps[:], ACTF.Copy)
                    lg = sb.tile([P, 2 * HEADS], F32, tag="lg", name="lg")
                    DOT_DT = F32 if os.environ.get("KB_F32DOT") else BF
                    scr = sb.tile([P, HEADS, C], DOT_DT, tag="scr", name="scr")
                    for h in range(HEADS):
                        nc.vector.scalar_tensor_tensor(
                            out=scr[:, h, :], in0=qall[:, h * C:(h + 1) * C],
                            scalar=1.0, in1=kvb[:, ti, h * C:(h + 1) * C],
                            op0=ALU.bypass, op1=ALU.mult,
                            accum_out=lg[:, h:h + 1])
                    pe_prod = sb.tile([P, HEADS * EDGE_DIM], BF, tag="pep", name="pep")
                    nc.vector.tensor_mul(pe_prod[:], qall[:, HID:],
                                         aeb[:, ti, 2 * P:])
                    nc.vector.reduce_sum(
                        lg[:, HEADS:].rearrange("p (h o) -> p h o", o=1),
                        pe_prod[:].rearrange("p (h c) -> p h c", c=EDGE_DIM),
                        axis=AX.X)
                    lgt = sb.tile([P, HEADS], F32, tag="lgt", name="lgt")
                    nc.vector.tensor_add(lgt[:], lg[:, :HEADS], lg[:, HEADS:])
                    mov = sb.tile([P, MOV], BF, tag="mov", name="mov")
                    al = mov[:, HEADS * EDGE_DIM:HEADS * EDGE_DIM + HEADS]
                    nc.scalar.activation(al, lgt[:], ACTF.Exp,
                                         scale=1.0 / math.sqrt(C))
                    nc.vector.tensor_tensor(
                        out=mov[:, :HEADS * EDGE_DIM].rearrange(
                            "p (h c) -> p h c", c=EDGE_DIM),
                        in0=aeb[:, ti, 2 * P:].rearrange("p (h c) -> p h c", c=EDGE_DIM),
                        in1=al.rearrange("p (h o) -> p h o", o=1
                                         ).to_broadcast([P, HEADS, EDGE_DIM]),
                        op=ALU.mult)
                    nc.vector.tensor_tensor(
                        out=mov[:, HEADS * EDGE_DIM + HEADS:].rearrange(
                            "p (h c) -> p h c", c=C),
                        in0=kvb[:, ti, HID:].rearrange("p (h c) -> p h c", c=C),
                        in1=al.rearrange("p (h o) -> p h o", o=1
                                         ).to_broadcast([P, HEADS, C]),
                        op=ALU.mult)
                    BD = HEADS * EDGE_DIM + HEADS  # 68
                    if ti < T - 1:
                        nc.tensor.matmul(ps_all[:], aeb[:, ti, 0:P], mov[:],
                                         start=(ti == 0), stop=False)
                    else:
                        nc.tensor.matmul(ps_all[:, :BD], aeb[:, ti, 0:P],
                                         mov[:, :BD], start=(ti == 0), stop=True)
                        nc.tensor.matmul(ps_all[:, BD:], aeb[:, ti, 0:P],
                                         mov[:, BD:], start=(ti == 0), stop=False)
                # ---- group tail: B @ We_blk, den copy ----
                BD = HEADS * EDGE_DIM + HEADS
                bsb = sb.tile([P, HEADS * EDGE_DIM], BF, tag="bsb", name="bsb")
                nc.scalar.activation(bsb[:], ps_all[:, :HEADS * EDGE_DIM], ACTF.Copy)
                nc.scalar.activation(den_sb[:, g * HEADS:(g + 1) * HEADS],
                                     ps_all[:, HEADS * EDGE_DIM:BD], ACTF.Copy)
                tp = psum.tile([HEADS * EDGE_DIM, P], BF, tag="tp", name="btp", bufs=1)
                nc.tensor.transpose(tp[:], bsb[:], ident[:])
                btsb = sb.tile([HEADS * EDGE_DIM, P], BF, tag="btsb", name="btsb")
                nc.scalar.activation(btsb[:], tp[:], ACTF.Copy)
                nc.tensor.matmul(ps_all[:, BD:], btsb[:], Web_sb[:],
                                 start=False, stop=True)

                # ---- fused node phase A: normalize, beta-gate, BN sums ----
                if SKIP_A:
                    nc.scalar.activation(xnew[:, g * HID:(g + 1) * HID],
                                         ps_all[:, BD:], ACTF.Copy)
                    toff += T
                    continue
                den = den_sb[:, g * HEADS:(g + 1) * HEADS]
                dmx = sb.tile([P, HEADS], F32, tag="dmx", name="dmx")
                nc.vector.tensor_scalar_max(dmx[:], den, 1e-30)
                rden = sb.tile([P, HEADS], F32, tag="rden", name="rden")
                nc.vector.reciprocal(rden[:], dmx[:])
                outn = sb.tile([P, HID], BF, tag="outn", name="outn")
                nc.vector.tensor_tensor(
                    out=outn[:].rearrange("p (h c) -> p h c", c=C),
                    in0=ps_all[:, BD:].rearrange("p (h c) -> p h c", c=C),
                    in1=rden[:].rearrange("p (h o) -> p h o", o=1
                                          ).to_broadcast([P, HEADS, C]),
                    op=ALU.mult)
                xps = proj_psum(g, 3)  # x_r = x@Wskip + bskip
                xr = sb.tile([P, HID], BF, tag="xr", name="xr")
                nc.scalar.activation(xr[:], xps[:, :HID], ACTF.Copy)
                bl = sb.tile([P, 2], F32, tag="bl", name="bl")
                scrb = sb.tile([P, HID], BF, tag="scrb", name="scrb")
                nc.vector.scalar_tensor_tensor(
                    out=scrb[:], in0=outn[:], scalar=1.0, in1=wa_sb[:],
                    op0=ALU.bypass, op1=ALU.mult, accum_out=bl[:, 0:1])
                nc.vector.scalar_tensor_tensor(
                    out=scrb[:], in0=xr[:], scalar=1.0, in1=wb_sb[:],
                    op0=ALU.bypass, op1=ALU.mult, accum_out=bl[:, 1:2])
                blsum = sb.tile([P, 1], F32, tag="blsum", name="blsum")
                nc.vector.tensor_add(blsum[:], bl[:, 0:1], bl[:, 1:2])
                bsig = sb.tile([P, 1], F32, tag="bsig", name="bsig")
                nc.scalar.activation(bsig[:], blsum[:], ACTF.Sigmoid)
                diff = sb.tile([P, HID], BF, tag="diff", name="diff")
                nc.vector.tensor_sub(diff[:], xr[:], outn[:])
                bd_t = sb.tile([P, HID], BF, tag="bd", name="bd")
                nc.vector.tensor_scalar_mul(bd_t[:], diff[:], bsig[:, 0:1])
                xn = xnew[:, g * HID:(g + 1) * HID]
                nc.vector.tensor_add(xn, outn[:], bd_t[:])
                nc.vector.tensor_scalar_mul(xn, xn, mct_sb[:, g:g + 1])
                x2 = sb.tile([P, HID], BF, tag="x2", name="x2")
                nc.vector.tensor_mul(x2[:], xn, xn)
                ci, c_first, c_last = ch_of[g]
                if c_first:
                    cur_ps[0] = psum.tile([1, 2 * HID], F32, tag="stats",
                                          name=f"stps{ci}", bufs=2)
                ps_st = cur_ps[0]
                nc.tensor.matmul(ps_st[:, :HID], ones_col[:], xn,
                                 start=c_first, stop=c_last)
                nc.tensor.matmul(ps_st[:, HID:], ones_col[:], x2[:],
                                 start=c_first, stop=c_last)
                if c_last:
                    nc.scalar.activation(
                        st_part[:, ci * 2 * HID:(ci + 1) * 2 * HID],
                        ps_st[:], ACTF.Copy)
                toff += T
            nc.leave_named_scope(f"edge{l}", _sc_edge, False)

            if KB_DUMP:
                for tt_ in range(NG):
                    dtmp2 = sb.tile([P, HID], F32, tag="dtmp", name="dtmp2")
                    nc.vector.tensor_copy(dtmp2[:], xnew[:, tt_ * HID:(tt_ + 1) * HID])
                    nc.sync.dma_start(
                        out=d_xn[:, (l * NG + tt_) * HID:(l * NG + tt_ + 1) * HID],
                        in_=dtmp2[:])
                if l == 0:
                    nc.sync.dma_start(out=d_den[:], in_=den_sb[:])

            # ---- BN stats all-reduce + finalize ----
            if "S" in KB_SKIP:
                continue
            _sc_bn = nc.enter_named_scope(f"bnst{l}", False)[0]
            st_fin = sb.tile([1, 2 * HID], F32, tag="stfin", name="stfin")
            if SKIP_A or not active:
                nc.vector.tensor_copy(st_fin[:], cur_ps[0][:])
            elif NCH == 1:
                nc.vector.tensor_copy(st_fin[:], st_part[:, 0:2 * HID])
            else:
                nc.vector.tensor_add(st_fin[:], st_part[:, 0:2 * HID],
                                     st_part[:, 2 * HID:4 * HID])
                for ci in range(2, NCH):
                    nc.vector.tensor_add(
                        st_fin[:], st_fin[:],
                        st_part[:, ci * 2 * HID:(ci + 1) * 2 * HID])
            st_a = sb.tile([1, HID], F32, tag="stsa", name="stsa")
            nc.vector.tensor_copy(st_a[:], st_fin[:, :HID])
            st_b = sb.tile([1, HID], F32, tag="stsb", name="stsb")
            nc.vector.tensor_copy(st_b[:], st_fin[:, HID:])
            nc.sync.dma_start(out=st_in[0:1, :], in_=st_a[:])
            nc.sync.dma_start(out=st_in[1:2, :], in_=st_b[:])
            if SIM1:
                nc.sync.dma_start(out=st_out[:], in_=st_in[:])
            else:
                nc.gpsimd.collective_compute("AllReduce", ALU.add,
                                             replica_groups=cg,
                                             ins=[st_in[:].opt()], outs=[st_out[:].opt()])
            str_a = sb.tile([1, HID], F32, tag="stra", name="stra")
            nc.sync.dma_start(out=str_a[:], in_=st_out[0:1, :])
            str_b = sb.tile([1, HID], F32, tag="strb", name="strb")
            nc.sync.dma_start(out=str_b[:], in_=st_out[1:2, :])
            mean = sb.tile([1, HID], F32, tag="mean", name="mean")
            nc.vector.tensor_scalar_mul(mean[:], str_a[:], 1.0 / N)
            var = sb.tile([1, HID], F32, tag="var", name="var")
            nc.vector.tensor_scalar_mul(var[:], str_b[:], 1.0 / N)
            msq = sb.tile([1, HID], F32, tag="msq", name="msq")
            nc.vector.tensor_mul(msq[:], mean[:], mean[:])
            nc.vector.tensor_sub(var[:], var[:], msq[:])
            nc.vector.tensor_scalar_add(var[:], var[:], EPS)
            sd = sb.tile([1, HID], F32, tag="sd", name="sd")
            nc.scalar.activation(sd[:], var[:], ACTF.Sqrt)
            rstd = sb.tile([1, HID], F32, tag="rstd", name="rstd")
            nc.vector.reciprocal(rstd[:], sd[:])
            # one Newton step for 1/sqrt(var): r <- r*(1.5 - 0.5*var*r^2)
            # (the ACT Sqrt table is only ~0.3% accurate; this error is a
            # per-channel systematic scale on x that compounds across layers)
            nt = sb.tile([1, HID], F32, tag="nt", name="nt")
            nc.vector.tensor_mul(nt[:], rstd[:], rstd[:])
            nc.vector.tensor_mul(nt[:], nt[:], var[:])
            nc.vector.tensor_scalar_mul(nt[:], nt[:], -0.5)
            nc.vector.tensor_scalar_add(nt[:], nt[:], 1.5)
            nc.vector.tensor_mul(rstd[:], rstd[:], nt[:])
            gam_sb = sb.tile([1, HID], F32, tag="gamsb", name="gamsb")
            nc.sync.dma_start(out=gam_sb[:], in_=gam[l][:])
            bet_sb = sb.tile([1, HID], F32, tag="betsb", name="betsb")
            nc.sync.dma_start(out=bet_sb[:], in_=bet[l][:])
            aa = sb.tile([1, HID], BF, tag="aa", name="aa")
            nc.vector.tensor_mul(aa[:], gam_sb[:], rstd[:])
            ac_ = sb.tile([1, HID], F32, tag="acs", name="acs")
            nc.vector.tensor_mul(ac_[:], mean[:], aa[:])
            ccs = sb.tile([1, HID], BF, tag="ccs", name="ccs")
            nc.vector.tensor_sub(ccs[:], bet_sb[:], ac_[:])
            for h in range(2):
                tpa = psum.tile([P, 2], F32, tag="tp", name="tpa", bufs=1)
                nc.tensor.matmul(tpa[:, 0:1], aa[0:1, h * P:(h + 1) * P], one1[:],
                                 start=True, stop=True)
                nc.tensor.matmul(tpa[:, 1:2], ccs[0:1, h * P:(h + 1) * P], one1[:],
                                 start=True, stop=True)
                nc.vector.tensor_copy(aT_sb[:, h:h + 1], tpa[:, 0:1])
                nc.vector.tensor_copy(cT_sb[:, h:h + 1], tpa[:, 1:2])
            nc.leave_named_scope(f"bnst{l}", _sc_bn, False)

            # ---- node phase B: transpose + BN apply + LeakyReLU -> x_T ----
            if l < L - 1 and "B" in KB_SKIP:
                load_layer_weights(l + 1)
                qkv_phase()
            elif l < L - 1:
                _sc_nB = nc.enter_named_scope(f"nodeB{l}", False)[0]
                mrep = sb.tile([P, P], BF, tag="mrep", name="mrep")
                nc.sync.dma_start(out=mrep[:], in_=maskrep[:])
                for t in range(NG):
                    for h in range(2):
                        tp = psum.tile([P, P], BF, tag="tp", name="xtp", bufs=1)
                        nc.tensor.transpose(
                            tp[:], xnew[:, t * HID + h * P:t * HID + (h + 1) * P],
                            ident[:])
                        ybn = sb.tile([P, P], BF, tag="ybn2", name="ybn2")
                        nc.scalar.activation(ybn[:], tp[:],
                                             ACTF.Identity,
                                             bias=cT_sb[:, h:h + 1],
                                             scale=aT_sb[:, h:h + 1])
                        y1 = sb.tile([P, P], BF, tag="y1b", name="y1b")
                        nc.vector.tensor_scalar_mul(y1[:], ybn[:], 0.1)
                        dst = x_T[h][:, t * P:(t + 1) * P]
                        nc.vector.tensor_max(dst, ybn[:], y1[:])
                        if t == NG - 1:
                            nc.vector.tensor_mul(dst, dst, mrep[:])
                nc.leave_named_scope(f"nodeB{l}", _sc_nB, False)
                if KB_DUMP and l == 0:
                    for h in range(2):
                        for tt_ in range(NG):
                            dxt = sb.tile([P, P], F32, tag="dxt", name="dxt", bufs=2)
                            nc.vector.tensor_copy(dxt[:], x_T[h][:, tt_ * P:(tt_ + 1) * P])
                            nc.sync.dma_start(
                                out=d_xt[:, h * n_pad + tt_ * P:h * n_pad + (tt_ + 1) * P],
                                in_=dxt[:])
                    dbn = sb.tile([1, 2 * HID], F32, tag="dbn", name="dbn")
                    nc.vector.tensor_copy(dbn[:, :HID], aa[:])
                    nc.vector.tensor_copy(dbn[:, HID:], ccs[:])
                    nc.sync.dma_start(out=d_bn[:], in_=dbn[:])
                load_layer_weights(l + 1)
                qkv_phase()
            else:
                # ---- pooling + head (x of last layer = BN+lrelu of xnew) ----
                _sc_pool = nc.enter_named_scope("pool", False)[0]
                arep_ps = psum.tile([P, QW], F32, tag="mm", name="arep")
                nc.tensor.matmul(arep_ps[:, :HID], ones_row[:], aa[:], start=True, stop=True)
                arep = sb.tile([P, HID], BF, tag="arep", name="arepsb")
                nc.vector.tensor_copy(arep[:], arep_ps[:, :HID])
                crep_ps = psum.tile([P, QW], F32, tag="mm", name="crep")
                nc.tensor.matmul(crep_ps[:, :HID], ones_row[:], ccs[:], start=True, stop=True)
                crep = sb.tile([P, HID], BF, tag="crep", name="crepsb")
                nc.vector.tensor_copy(crep[:], crep_ps[:, :HID])
                ps_pool = psum.tile([G, 1], F32, tag="stats", name="poolps", bufs=2)
                if "pool" in KB_SKIP:
                    ptd = sb.tile([P, G], BF, tag="ptsb", name="ptd")
                    nc.vector.memset(ptd[:], 0.0)
                    nc.tensor.matmul(ps_pool[:], ptd[:], ones_col[:, 0:1],
                                     start=True, stop=True)
                for t in range(NG if "pool" not in KB_SKIP else 0):
                    xn = xnew[:, t * HID:(t + 1) * HID]
                    y1 = sb.tile([P, HID], BF, tag="y1", name="y1")
                    nc.vector.tensor_mul(y1[:], xn, arep[:])
                    ybn = sb.tile([P, HID], BF, tag="ybn", name="ybn")
                    nc.vector.tensor_add(ybn[:], y1[:], crep[:])
                    yr = sb.tile([P, HID], BF, tag="yr", name="yr")
                    # leaky relu: max(x, 0.1x)
                    nc.vector.tensor_scalar_mul(y1[:], ybn[:], 0.1)
                    nc.vector.tensor_max(yr[:], ybn[:], y1[:])
                    s1 = sb.tile([P, 2], F32, tag="s1", name="s1")
                    scrb2 = sb.tile([P, HID], BF, tag="scrb", name="scrb2")
                    nc.vector.scalar_tensor_tensor(
                        out=scrb2[:], in0=yr[:], scalar=1.0, in1=h1r_sb[:],
                        op0=ALU.bypass, op1=ALU.mult, accum_out=s1[:, 0:1])
                    nc.vector.scalar_tensor_tensor(
                        out=scrb2[:], in0=yr[:], scalar=1.0, in1=h2r_sb[:],
                        op0=ALU.bypass, op1=ALU.mult, accum_out=s1[:, 1:2])
                    yv = sb.tile([P, 1], F32, tag="yv", name="yv")
                    nc.vector.tensor_mul(yv[:], s1[:, 0:1], ict_sb[:, t:t + 1])
                    yw = sb.tile([P, 1], BF, tag="yw", name="yw")
                    nc.vector.tensor_add(yw[:], yv[:], s1[:, 1:2])
                    # mask pads (bn shifts pads off zero)
                    nc.vector.tensor_scalar_mul(yw[:], yw[:], mct_sb[:, t:t + 1])
                    pt_sb = sb.tile([P, G], BF, tag="ptsb", name="ptsb")
                    nc.sync.dma_start(out=pt_sb[:], in_=p_t[t * P:(t + 1) * P, :])
                    nc.tensor.matmul(ps_pool[:], pt_sb[:], yw[:],
                                     start=(t == 0), stop=(t == NG - 1))
                pool_sb = sb.tile([G, 1], F32, tag="poolsb", name="poolsb")
                nc.vector.tensor_copy(pool_sb[:], ps_pool[:])
                nc.sync.dma_start(out=pool_in[:], in_=pool_sb[:])
                if SIM1:
                    nc.sync.dma_start(out=pool_out[:], in_=pool_in[:])
                else:
                    nc.gpsimd.collective_compute("AllReduce", ALU.add,
                                                 replica_groups=cg,
                                                 ins=[pool_in[:].opt()],
                                                 outs=[pool_out[:].opt()])
                pr = sb.tile([G, 1], F32, tag="pr", name="pr")
                nc.sync.dma_start(out=pr[:], in_=pool_out[:])
                fin = sb.tile([G, 1], F32, tag="fin", name="fin")
                nc.vector.tensor_scalar_add(fin[:], pr[:], float(head_b))
                nc.sync.dma_start(out=out_ext[:], in_=fin[:])
                nc.leave_named_scope("pool", _sc_pool, False)

        ctx.close()
    return nc


LAST_RESULT = None


def kernel(**inputs):
    global LAST_RESULT
    meta, cores = plan(inputs["edge_index"], inputs["batch"])
    in_maps = build_inmaps(inputs, meta, cores)
    head_b = float(np.asarray(inputs["head_b"]).reshape(-1)[0])
    nc = bacc.Bacc("TRN2")
    build(nc, meta, head_b)
    if not nc.is_finalized():
        nc.finalize()
    res = run_bass_kernel_spmd(nc, in_maps, core_ids=list(range(NCORES)))
    LAST_RESULT = res
    out = np.asarray(res.results[0]["out"], dtype=np.float32).reshape(G)
    return out


if __name__ == "__main__":
    import reference
    inputs = {k: np.asarray(v) for k, v in reference.setup_inputs().items()}
    got = kernel(**inputs)
    exp = np.asarray(reference.reference(**inputs))
    rel = np.abs(got - exp).max() / (np.abs(exp).max() + 1e-9)
    print("Relative error:", rel)


# revision 46
# speedup vs baseline: 1.7778x; 1.7778x over previous
"""Trainium2 Bass kernel for AnticipatoryRestaurantGNN (TransformerConv x4 + BN + pool).

Strategy (edge-parallel, dst-sorted):
  - Sort edges by dst; partition nodes into 8 contiguous ranges with ~equal
    edge counts. Each core owns its node range and ALL edges pointing into it,
    so segment-softmax and scatter-add are core-local.
  - Per layer, each core computes q/k/v projections for its own nodes only;
    k/v (bf16) are AllGathered so every core can gather k[src], v[src] rows
    for its edge shard with one batched indirect DMA per 128-dst node group.
  - Edge-attr projection e = ea@We is never materialized per edge:
      logit = q.(k+e) = q.k + qE.ea       with qE = x @ (Wq_h We_h^T) (folded
                                          into the q projection, 64 extra cols)
      out   = sum a(v+e) = sum a v  +  B @ We_blk   with B = at^T @ (a (x) ea)
    accumulated per dst-group on the TensorEngine.
  - Softmax uses exp(logit) directly (shift-invariant; logits are O(1) here):
    accumulate sum(alpha*v), B, and sum(alpha) per dst node via host-baked
    one-hot scatter matmuls (PSUM accumulate), normalized in the node phase.
  - BatchNorm stats and the final pooled head are AllReduced (tiny).
  - Per-edge-tile work on DVE is minimized: fused multiply+accumulate dots
    (scalar_tensor_tensor with accum_out), PSUM->SBUF staging on the (idle)
    Activation engine, per-group batched DMA for gather/one-hots/edge-attrs.
"""

import heapq
import math
import os
import sys

sys.path.insert(0, "/opt/trn_rl_repo")

import ml_dtypes
import numpy as np

import concourse.bacc as bacc
import concourse.bass as bass
import concourse.mybir as mybir
import concourse.tile as tile
from concourse.bass_utils import run_bass_kernel_spmd
from concourse.masks import make_identity

BF16 = ml_dtypes.bfloat16

N, E, IN_DIM, EDGE_DIM, HID, L, HEADS, G = 50000, 500000, 64, 16, 256, 4, 4, 64
C = HID // HEADS
NCORES = 8
P = 128
EPS = 1e-5
QW = HID + HEADS * EDGE_DIM  # 320: q | qE
MOV = HID + HEADS * EDGE_DIM + HEADS  # 324: [alpha*ea(64) | alpha(4) | alpha*v(256)]

F32 = mybir.dt.float32
BF = mybir.dt.bfloat16

AX = mybir.AxisListType
ALU = mybir.AluOpType
ACTF = mybir.ActivationFunctionType


def _roundup(x, m):
    return (x + m - 1) // m * m


def _tctile(tc, *a, **kw):
    t, _free = tc.tile(*a, **kw)
    return t


def plan(edge_index, batch):
    """Host-side layout planning. Returns (meta, per_core_arrays)."""
    src, dst = np.asarray(edge_index[0]), np.asarray(edge_index[1])
    batch = np.asarray(batch)

    order = np.argsort(dst, kind="stable")
    s_src = src[order].astype(np.int64)
    s_dst = dst[order].astype(np.int64)

    deg = np.bincount(dst, minlength=N)
    cum = np.concatenate([[0], np.cumsum(deg)])  # cum[n] = first edge of node n

    # node range split, balanced by edge count, at node boundaries
    ns = [0]
    for i in range(1, NCORES):
        tgt = round(E * i / NCORES)
        ns.append(int(np.searchsorted(cum, tgt, side="left")))
    ns.append(N)
    ns = np.array(ns, dtype=np.int64)
    n_own = np.diff(ns)
    n_pad = _roundup(int(n_own.max()), P)
    NG = n_pad // P
    ROWS = NCORES * n_pad
    BANKN = ROWS // 2
    assert BANKN <= 32767

    core_of = np.searchsorted(ns[1:], np.arange(N), side="right")

    # LPT node->group assignment per core: balance per-group edge counts so
    # every group needs the same number of 128-edge tiles (fewer padded tiles)
    loc_of = np.zeros(N, dtype=np.int64)   # core-local padded row of each node
    group_nodes_all = []
    for c in range(NCORES):
        nodes = np.arange(ns[c], ns[c + 1])
        degs = deg[nodes]
        order_d = np.argsort(-degs, kind="stable")
        loads = np.zeros(NG, dtype=np.int64)
        fill = np.zeros(NG, dtype=np.int64)
        gnodes = [[] for _ in range(NG)]
        heap = [(0, g) for g in range(NG)]
        heapq.heapify(heap)
        for i in order_d:
            while True:
                _, g = heapq.heappop(heap)
                if fill[g] < P:
                    break
            gnodes[g].append(int(nodes[i]))
            fill[g] += 1
            loads[g] += int(degs[i])
            heapq.heappush(heap, (int(loads[g]), g))
        for g in range(NG):
            for j, n in enumerate(gnodes[g]):
                loc_of[n] = g * P + j
        group_nodes_all.append(gnodes)

    padrow = core_of * n_pad + loc_of
    src_p = padrow[s_src]  # padded global row per sorted edge

    # per (core, group) edge lists (edges sorted by src for HBM locality)
    per_core_ed = []
    Tmax = np.zeros(NG, dtype=np.int64)
    for c in range(NCORES):
        groups = []
        for g in range(NG):
            gn = group_nodes_all[c][g]
            if gn:
                eidx = np.concatenate([np.arange(cum[n], cum[n + 1]) for n in gn])
            else:
                eidx = np.arange(0, 0)
            eidx = eidx[np.argsort(src_p[eidx], kind="stable")]
            groups.append(eidx)
            Tmax[g] = max(Tmax[g], _roundup(len(eidx), P) // P)
        per_core_ed.append(groups)

    TOTE = int(Tmax.sum()) * P  # padded edges per core (same on all cores)

    counts = np.bincount(batch, minlength=G).astype(np.float64)

    meta = dict(ns=ns, n_pad=n_pad, NG=NG, ROWS=ROWS, BANKN=BANKN,
                Tmax=Tmax, TOTE=TOTE, order=order, counts=counts)

    cores = []
    for c in range(NCORES):
        kv_idx = np.zeros(TOTE, dtype=np.int32)
        at_all = np.zeros((TOTE, P), dtype=np.float32)
        ea_sel = np.full(TOTE, -1, dtype=np.int64)
        off = 0
        for g in range(NG):
            el = per_core_ed[c][g]
            T = int(Tmax[g])
            if T == 0:
                continue
            npad = T * P
            k = len(el)
            kvv = np.zeros(npad, dtype=np.int64)
            kvv[:k] = src_p[el]
            kv_idx[off:off + npad] = kvv.astype(np.int32)
            dr = np.full(npad, -1, dtype=np.int64)
            if k:
                dr[:k] = loc_of[s_dst[el]] - g * P
            valid = dr >= 0
            atb = np.zeros((npad, P), dtype=np.float32)
            atb[np.arange(npad)[valid], dr[valid]] = 1.0
            at_all[off:off + npad] = atb
            ea_sel[off:off + k] = el
            off += npad
        assert off == TOTE

        # kv gather indices: [128, TT] column-per-tile (idx j -> [j%128, j//128])
        TT = TOTE // P
        kv_idx32 = np.ascontiguousarray(kv_idx.reshape(TT, P).T)

        # fused per-edge operand block: [at(128) | at^T(128) | ea4 slot(64)]
        acat = np.zeros((TT, P, 2 * P + HEADS * EDGE_DIM), dtype=BF16)
        for t in range(TT):
            blk = at_all[t * P:(t + 1) * P]
            acat[t, :, 0:P] = blk.astype(BF16)
            acat[t, :, P:2 * P] = blk.T.astype(BF16)

        nn = int(n_own[c])
        nodes = np.arange(ns[c], ns[c + 1])
        ll = loc_of[nodes]
        flat_inv = np.zeros(n_pad, dtype=np.float32)
        flat_inv[ll] = 1.0 / np.maximum(counts[batch[nodes]], 1.0)
        flat_msk = np.zeros(n_pad, dtype=np.float32)
        flat_msk[ll] = 1.0
        invcnt_t = np.ascontiguousarray(flat_inv.reshape(NG, P).T)
        maskcol_t = np.ascontiguousarray(flat_msk.reshape(NG, P).T)
        p_t = np.zeros((n_pad, G), dtype=np.float32)
        p_t[ll, batch[nodes]] = 1.0
        maskrep = np.repeat(flat_msk[(NG - 1) * P:NG * P][None, :], P, axis=0)

        cores.append(dict(kv_idx32=kv_idx32, acat=acat, ea_sel=ea_sel,
                          invcnt_t=invcnt_t, maskcol_t=maskcol_t,
                          maskrep=maskrep.astype(BF16), p_t=p_t.astype(BF16),
                          nn=nn, ll=ll))
    return meta, cores


def build_inmaps(inputs, meta, cores):
    """Build the per-core in_maps dict for run_bass_kernel_spmd."""
    ns, n_pad, NG = meta["ns"], meta["n_pad"], meta["NG"]
    order = meta["order"]
    ea_sorted = np.asarray(inputs["edge_attr"], dtype=np.float32)[order]

    def f32(x):
        return np.ascontiguousarray(np.asarray(x, dtype=np.float32))

    def bf(x):
        return np.ascontiguousarray(np.asarray(x, dtype=np.float32).astype(BF16))

    nf = np.asarray(inputs["node_features"])
    Wbeta = np.asarray(inputs["Wbeta"])  # [L, 768, 1]
    head_W = np.asarray(inputs["head_W"])  # [512, 1]

    shared = {}
    shared["proj_W"] = bf(inputs["proj_W"])                      # [64,256]
    shared["proj_b"] = bf(inputs["proj_b"][None, :])             # [1,256]
    for l in range(L):
        Wq = np.asarray(inputs["Wq"][l], dtype=np.float64)
        We = np.asarray(inputs["We"][l], dtype=np.float64)       # [16, 256]
        bq = np.asarray(inputs["bq"][l], dtype=np.float64)
        # q projection extended with qE_h = q_h @ We_h^T (folds e into logit)
        Wq_p = np.zeros((HID, QW), dtype=np.float64)
        bq_p = np.zeros(QW, dtype=np.float64)
        Wq_p[:, :HID] = Wq
        bq_p[:HID] = bq
        for h in range(HEADS):
            We_h = We[:, h * C:(h + 1) * C]                      # [16, 64]
            Wq_p[:, HID + h * EDGE_DIM:HID + (h + 1) * EDGE_DIM] = \
                Wq[:, h * C:(h + 1) * C] @ We_h.T
            bq_p[HID + h * EDGE_DIM:HID + (h + 1) * EDGE_DIM] = \
                bq[h * C:(h + 1) * C] @ We_h.T
        shared[f"Wq{l}"] = bf(Wq_p)
        shared[f"bq{l}"] = bf(bq_p[None, :])
        shared[f"Wk{l}"] = bf(inputs["Wk"][l])
        shared[f"Wv{l}"] = bf(inputs["Wv"][l])
        shared[f"Wsk{l}"] = bf(inputs["Wskip"][l])
        shared[f"bk{l}"] = bf((inputs["bk"][l] + inputs["be"][l])[None, :])
        shared[f"bv{l}"] = bf((inputs["bv"][l] + inputs["be"][l])[None, :])
        shared[f"bsk{l}"] = bf(inputs["bskip"][l][None, :])
        # block-diagonal We for the per-group B @ We_blk edge contribution
        We_blk = np.zeros((HEADS * EDGE_DIM, HID), dtype=np.float64)
        for h in range(HEADS):
            We_blk[h * EDGE_DIM:(h + 1) * EDGE_DIM, h * C:(h + 1) * C] = \
                We[:, h * C:(h + 1) * C]
        shared[f"Web{l}"] = bf(We_blk)
        w1, w2, w3 = Wbeta[l, :HID, 0], Wbeta[l, HID:2 * HID, 0], Wbeta[l, 2 * HID:, 0]
        shared[f"wa{l}"] = bf(np.repeat((w1 + w3)[None, :], P, axis=0))   # [128,256]
        shared[f"wb{l}"] = bf(np.repeat((w2 - w3)[None, :], P, axis=0))
        shared[f"gam{l}"] = f32(inputs["bn_gamma"][l][None, :])
        shared[f"bet{l}"] = f32(inputs["bn_beta"][l][None, :])
    shared["h1r"] = bf(np.repeat(head_W[:HID, 0][None, :], P, axis=0))
    shared["h2r"] = bf(np.repeat(head_W[HID:, 0][None, :], P, axis=0))

    in_maps = []
    for c in range(NCORES):
        cc = cores[c]
        m = dict(shared)
        nft = np.zeros((IN_DIM, n_pad), dtype=np.float32)
        nft[:, cc["ll"]] = nf[ns[c]:ns[c + 1]].T
        m["nf_t"] = nft.astype(BF16)
        # fill ea4 slot of acat: ea tiled x4 heads
        acat = cc["acat"].copy()
        sel = cc["ea_sel"]
        ea_full = np.zeros((meta["TOTE"], EDGE_DIM), dtype=np.float32)
        ea_full[sel >= 0] = ea_sorted[sel[sel >= 0]]
        ea4 = np.tile(ea_full, (1, HEADS)).astype(BF16)          # [TOTE, 64]
        acat[:, :, 2 * P:] = ea4.reshape(meta["TOTE"] // P, P, HEADS * EDGE_DIM)
        m["acat"] = np.ascontiguousarray(acat.reshape(meta["TOTE"], 2 * P + HEADS * EDGE_DIM))
        m["kv_idx32"] = cc["kv_idx32"]                           # [128, TT] int32
        m["invcnt_t"] = cc["invcnt_t"]
        m["maskcol_t"] = cc["maskcol_t"]
        m["maskrep"] = cc["maskrep"]
        m["p_t"] = cc["p_t"]
        in_maps.append(m)
    return in_maps


def build(nc, meta, head_b):
    n_pad, NG, ROWS, Tmax, TOTE = (meta["n_pad"], meta["NG"], meta["ROWS"],
                                   meta["Tmax"], meta["TOTE"])
    TT = TOTE // P
    TMX = int(Tmax.max())
    AC = 2 * P + HEADS * EDGE_DIM  # 320 cols of acat

    # ---- dram parameters ----
    dp = lambda name, shape, dt: nc.declare_dram_parameter(name, list(shape), dt, isOutput=False)
    nf_t = dp("nf_t", (IN_DIM, n_pad), BF)
    acat = dp("acat", (TOTE, AC), BF)
    kv_idx32 = dp("kv_idx32", (P, TT), mybir.dt.int32)
    invcnt_t = dp("invcnt_t", (P, NG), F32)
    maskcol_t = dp("maskcol_t", (P, NG), F32)
    maskrep = dp("maskrep", (P, P), BF)
    p_t = dp("p_t", (n_pad, G), BF)
    proj_W = dp("proj_W", (IN_DIM, HID), BF)
    proj_b = dp("proj_b", (1, HID), BF)
    Wq, Wk, Wv, Wsk, bq, bk, bv, bsk, Web, wa, wb, gam, bet = ({} for _ in range(13))
    for l in range(L):
        Wq[l] = dp(f"Wq{l}", (HID, QW), BF)
        Wk[l] = dp(f"Wk{l}", (HID, HID), BF)
        Wv[l] = dp(f"Wv{l}", (HID, HID), BF)
        Wsk[l] = dp(f"Wsk{l}", (HID, HID), BF)
        bq[l] = dp(f"bq{l}", (1, QW), BF)
        bk[l] = dp(f"bk{l}", (1, HID), BF)
        bv[l] = dp(f"bv{l}", (1, HID), BF)
        bsk[l] = dp(f"bsk{l}", (1, HID), BF)
        Web[l] = dp(f"Web{l}", (HEADS * EDGE_DIM, HID), BF)
        wa[l] = dp(f"wa{l}", (P, HID), BF)
        wb[l] = dp(f"wb{l}", (P, HID), BF)
        gam[l] = dp(f"gam{l}", (1, HID), F32)
        bet[l] = dp(f"bet{l}", (1, HID), F32)
    h1r = dp("h1r", (P, HID), BF)
    h2r = dp("h2r", (P, HID), BF)
    out_ext = nc.declare_dram_parameter("out", [G, 1], F32, isOutput=True)
    KB_DUMP = os.environ.get("KB_DUMP", "") == "1"
    if KB_DUMP:
        d_den = nc.declare_dram_parameter("d_den", [P, NG * HEADS], F32, isOutput=True)
        d_xn = nc.declare_dram_parameter("d_xn", [P, L * NG * HID], F32, isOutput=True)
        d_xt = nc.declare_dram_parameter("d_xt", [P, 2 * n_pad], F32, isOutput=True)
        d_bn = nc.declare_dram_parameter("d_bn", [1, 2 * HID], F32, isOutput=True)

    cg = [list(range(NCORES))]
    SIM1 = os.environ.get("KB_SIM1", "") == "1"

    # WW stationary layout offsets: (col0, width, bias0)
    WOFF = {0: (0, QW, 0), 1: (2 * QW, HID, QW), 2: (2 * QW + 2 * HID, HID, QW + HID),
            3: (2 * QW + 4 * HID, HID, QW + 2 * HID)}
    WWC = 2 * QW + 6 * HID
    BWC = QW + 3 * HID

    with tile.TileContext(nc) as tc:
        # ---- persistent DRAM ----
        kv_slab = _tctile(tc, [n_pad, 2 * HID], BF, space="DRAM", name="kv_slab")
        q_slab = _tctile(tc, [n_pad, QW], BF, space="DRAM", name="q_slab")
        kv_full = _tctile(tc, [ROWS, 2 * HID], BF, space="DRAM", addr_space="Shared", name="kv_full")
        st_in = _tctile(tc, [2, HID], F32, space="DRAM", name="st_in")
        st_out = _tctile(tc, [2, HID], F32, space="DRAM", addr_space="Shared", name="st_out")
        pool_in = _tctile(tc, [G, 1], F32, space="DRAM", name="pool_in")
        pool_out = _tctile(tc, [G, 1], F32, space="DRAM", addr_space="Shared", name="pool_out")

        # ---- persistent SBUF ----
        from contextlib import ExitStack
        ctx = ExitStack()
        res = ctx.enter_context(tc.tile_pool(name="res", bufs=1))
        x_T = [res.tile([P, n_pad], BF, tag=f"xT{h}", name=f"xT{h}") for h in range(2)]
        xnew = res.tile([P, NG * HID], BF, tag="xnew", name="xnew")
        den_sb = res.tile([P, NG * HEADS], F32, tag="densb", name="densb")
        ident = res.tile([P, P], BF, tag="ident", name="ident")
        make_identity(nc, ident[:])
        ones_row = res.tile([1, P], BF, tag="onesr", name="onesr")
        nc.vector.memset(ones_row[:], 1.0)
        ones_col = res.tile([P, 1], BF, tag="onesc", name="onesc")
        nc.vector.memset(ones_col[:], 1.0)
        one1 = res.tile([1, 1], BF, tag="one1", name="one1")
        nc.vector.memset(one1[:], 1.0)
        wa_sb = res.tile([P, HID], BF, tag="wasb", name="wasb")
        wb_sb = res.tile([P, HID], BF, tag="wbsb", name="wbsb")
        aT_sb = res.tile([P, 2], F32, tag="aTsb", name="aTsb")   # col h = scale for half h
        cT_sb = res.tile([P, 2], F32, tag="cTsb", name="cTsb")
        WW = res.tile([P, WWC], BF, tag="WW", name="WW")
        Web_sb = res.tile([HEADS * EDGE_DIM, HID], BF, tag="Websb", name="Websb")
        kvidx_sb = res.tile([P, TT], mybir.dt.int32, tag="kvidx", name="kvidx")
        nc.sync.dma_start(out=kvidx_sb[:], in_=kv_idx32[:])
        bias_sb = res.tile([1, BWC], BF, tag="biassb", name="biassb")
        mct_sb = res.tile([P, NG], F32, tag="mctsb", name="mctsb")
        nc.sync.dma_start(out=mct_sb[:], in_=maskcol_t[:])
        ict_sb = res.tile([P, NG], F32, tag="ictsb", name="ictsb")
        nc.sync.dma_start(out=ict_sb[:], in_=invcnt_t[:])

        h1r_sb = res.tile([P, HID], BF, tag="h1rsb", name="h1rsb")
        nc.sync.dma_start(out=h1r_sb[:], in_=h1r[:])
        h2r_sb = res.tile([P, HID], BF, tag="h2rsb", name="h2rsb")
        nc.sync.dma_start(out=h2r_sb[:], in_=h2r[:])

        KB_SKIP = set(os.environ.get("KB_SKIP", "").split(","))
        for h in range(2):
            nc.vector.memset(x_T[h][:], 0.01)
        nc.vector.memset(xnew[:], 0.0)
        nc.vector.memset(den_sb[:], 1.0)
        nc.vector.memset(aT_sb[:], 1.0)
        nc.vector.memset(cT_sb[:], 0.0)

        wpool = ctx.enter_context(tc.tile_pool(name="wtmp", bufs=1))
        psum = ctx.enter_context(tc.tile_pool(name="ps", bufs=2, space="PSUM"))
        sb = ctx.enter_context(tc.tile_pool(name="sb", bufs=3))

        def load_layer_weights(l):
            for i, W in enumerate((Wq[l], Wk[l], Wv[l], Wsk[l])):
                w0, w, _ = WOFF[i]
                for h in range(2):
                    nc.sync.dma_start(out=WW[:, w0 + h * w:w0 + (h + 1) * w],
                                      in_=W[h * P:(h + 1) * P, :])
            nc.sync.dma_start(out=Web_sb[:], in_=Web[l][:])
            for i, b in enumerate((bq[l], bk[l], bv[l], bsk[l])):
                _, w, b0 = WOFF[i]
                nc.sync.dma_start(out=bias_sb[:, b0:b0 + w], in_=b[:])
            nc.sync.dma_start(out=wa_sb[:], in_=wa[l][:])
            nc.sync.dma_start(out=wb_sb[:], in_=wb[l][:])

        def proj_psum(t, widx):
            """x_T node-tile t through weight widx (0..3 -> q,k,v,skip) -> PSUM."""
            w0, w, b0 = WOFF[widx]
            ps = psum.tile([P, QW], F32, tag="mm", name="projps")
            nc.tensor.matmul(ps[:, :w], x_T[0][:, t * P:(t + 1) * P],
                             WW[:, w0:w0 + w], start=True, stop=False)
            nc.tensor.matmul(ps[:, :w], x_T[1][:, t * P:(t + 1) * P],
                             WW[:, w0 + w:w0 + 2 * w], start=False, stop=False)
            nc.tensor.matmul(ps[:, :w], ones_row[:],
                             bias_sb[:, b0:b0 + w], start=False, stop=True)
            return ps

        def qkv_phase():
            """compute q/k/v slabs from x_T, all-gather k/v."""
            if "qkv" in KB_SKIP:
                return
            with nc.named_scope("qkv"):
                for t in range(NG):
                    qp = proj_psum(t, 0)
                    qrow = sb.tile([P, QW], BF, tag="qrow", name="qrow")
                    nc.scalar.activation(qrow[:], qp[:], ACTF.Copy)
                    nc.sync.dma_start(out=q_slab[t * P:(t + 1) * P, :], in_=qrow[:])
                    kvrow = sb.tile([P, 2 * HID], BF, tag="kvrow", name="kvrow")
                    kp = proj_psum(t, 1)
                    nc.scalar.activation(kvrow[:, :HID], kp[:, :HID], ACTF.Copy)
                    vp = proj_psum(t, 2)
                    nc.scalar.activation(kvrow[:, HID:], vp[:, :HID], ACTF.Copy)
                    nc.sync.dma_start(out=kv_slab[t * P:(t + 1) * P, :], in_=kvrow[:])
            with nc.named_scope("allgather"):
                if SIM1:
                    for c in range(NCORES):
                        nc.sync.dma_start(out=kv_full[c * n_pad:(c + 1) * n_pad, :],
                                          in_=kv_slab[:])
                else:
                    nc.gpsimd.collective_compute(
                        "AllGather", ALU.bypass, replica_groups=cg,
                        ins=[kv_slab[:].opt()], outs=[kv_full[:].opt()])

        # ================= init: x0 = nf @ proj_W + proj_b =================
        nfs = wpool.tile([IN_DIM, n_pad], BF, tag="nfs", name="nfs")
        nc.sync.dma_start(out=nfs[:], in_=nf_t[:])
        pw_sb = wpool.tile([IN_DIM, HID], BF, tag="pw", name="pw")
        nc.sync.dma_start(out=pw_sb[:], in_=proj_W[:])
        pb_sb = wpool.tile([1, HID], BF, tag="pb", name="pb")
        nc.sync.dma_start(out=pb_sb[:], in_=proj_b[:])
        for t in range(NG if "x0" not in KB_SKIP else 0):
            ps = psum.tile([P, QW], F32, tag="mm", name="x0ps")
            nc.tensor.matmul(ps[:, :HID], nfs[:, t * P:(t + 1) * P], pw_sb[:], start=True, stop=False)
            nc.tensor.matmul(ps[:, :HID], ones_row[:], pb_sb[:], start=False, stop=True)
            xr0 = sb.tile([P, HID], BF, tag="xrow0", name="xrow0")
            nc.scalar.activation(xr0[:], ps[:, :HID], ACTF.Copy)
            for h in range(2):
                tp = psum.tile([P, P], BF, tag="tp", name="x0tp", bufs=1)
                nc.tensor.transpose(tp[:], xr0[:, h * P:(h + 1) * P], ident[:])
                nc.scalar.activation(x_T[h][:, t * P:(t + 1) * P], tp[:], ACTF.Copy)

        load_layer_weights(0)
        qkv_phase()

        # ================= layers =================
        for l in range(L):
            # ---- edge phase (node phase A fused into each group tail) ----
            KB_NG = int(os.environ.get("KB_NG", "9999"))
            KB_L = int(os.environ.get("KB_L", "9999"))
            SKIP_A = "A" in KB_SKIP
            _sc_edge = nc.enter_named_scope(f"edge{l}", False)[0]
            active = [g for g in range(NG)
                      if Tmax[g] > 0 and g < KB_NG and l < KB_L]
            # chunked stats accumulation (two-level sum: less f32 error growth)
            NCH = int(os.environ.get("KB_NCH", "1"))
            st_part = sb.tile([1, 5 * 2 * HID], F32, tag="stpart",
                              name="stpart", bufs=1)
            nc.vector.memset(st_part[:], 0.0)
            chunks = [list(a) for a in np.array_split(np.array(active), NCH)]
            ch_of = {}
            for ci, ch in enumerate(chunks):
                for g in ch:
                    ch_of[g] = (ci, g == ch[0], g == ch[-1])
            cur_ps = [None]
            if SKIP_A or not active:
                cur_ps[0] = psum.tile([1, 2 * HID], F32, tag="stats",
                                      name="stps", bufs=2)
                nc.tensor.matmul(cur_ps[0][:, :HID], ones_col[:], xnew[:, 0:HID],
                                 start=True, stop=True)
            toff = 0  # global edge-tile counter
            for g in range(NG):
                T = int(Tmax[g])
                if g not in active:
                    if T:
                        toff += T
                    nc.vector.memset(xnew[:, g * HID:(g + 1) * HID], 0.0)
                    nc.vector.memset(den_sb[:, g * HEADS:(g + 1) * HEADS], 0.0)
                    continue
                qg_sb = sb.tile([P, QW], BF, tag="qgrp", name="qgrp", bufs=2)
                nc.sync.dma_start(out=qg_sb[:], in_=q_slab[g * P:(g + 1) * P, :])
                KVB_BUFS = 1 if KB_DUMP else 2
                kvb = sb.tile([P, TMX, 2 * HID], BF, tag="kvb", name="kvb",
                              bufs=KVB_BUFS)
                for ti in range(T):
                    nc.gpsimd.indirect_dma_start(
                        out=kvb[:, ti, :], out_offset=None, in_=kv_full[:],
                        in_offset=bass.IndirectOffsetOnAxis(
                            ap=kvidx_sb[:, toff + ti:toff + ti + 1], axis=0))
                aeb = sb.tile([P, TMX, AC], BF, tag="aeb", name="aeb", bufs=2)
                nc.sync.dma_start(
                    out=aeb[:, :T, :],
                    in_=acat[toff * P:(toff + T) * P, :].rearrange(
                        "(t p) c -> p t c", p=P))
                ps_all = psum.tile([P, MOV], F32, tag="acc", name="psall")
                for ti in range(T):
                    qps = psum.tile([P, QW], F32, tag="mm2", name="qps", bufs=1)
                    nc.tensor.matmul(qps[:], aeb[:, ti, P:2 * P], qg_sb[:],
                                     start=True, stop=True)
                    qall = sb.tile([P, QW], BF, tag="qall", name="qall")
                    nc.scalar.activation(qall[:], qps[:], ACTF.Copy)
                    lg = sb.tile([P, 2 * HEADS], F32, tag="lg", name="lg")
                    DOT_DT = F32 if os.environ.get("KB_F32DOT") else BF
                    scr = sb.tile([P, HEADS, C], DOT_DT, tag="scr", name="scr")
                    for h in range(HEADS):
                        nc.vector.scalar_tensor_tensor(
                            out=scr[:, h, :], in0=qall[:, h * C:(h + 1) * C],
                            scalar=1.0, in1=kvb[:, ti, h * C:(h + 1) * C],
                            op0=ALU.bypass, op1=ALU.mult,
                            accum_out=lg[:, h:h + 1])
                    pe_prod = sb.tile([P, HEADS * EDGE_DIM], BF, tag="pep", name="pep")
                    nc.vector.tensor_mul(pe_prod[:], qall[:, HID:],
                                         aeb[:, ti, 2 * P:])
                    nc.vector.reduce_sum(
                        lg[:, HEADS:].rearrange("p (h o) -> p h o", o=1),
                        pe_prod[:].rearrange("p (h c) -> p h c", c=EDGE_DIM),
                        axis=AX.X)
                    lgt = sb.tile([P, HEADS], F32, tag="lgt", name="lgt")
                    nc.vector.tensor_add(lgt[:], lg[:, :HEADS], lg[:, HEADS:])
                    mov = sb.tile([P, MOV], BF, tag="mov", name="mov")
                    al = mov[:, HEADS * EDGE_DIM:HEADS * EDGE_DIM + HEADS]
                    nc.scalar.activation(al, lgt[:], ACTF.Exp,
                                         scale=1.0 / math.sqrt(C))
                    nc.vector.tensor_tensor(
                        out=mov[:, :HEADS * EDGE_DIM].rearrange(
                            "p (h c) -> p h c", c=EDGE_DIM),
                        in0=aeb[:, ti, 2 * P:].rearrange("p (h c) -> p h c", c=EDGE_DIM),
                        in1=al.rearrange("p (h o) -> p h o", o=1
                                         ).to_broadcast([P, HEADS, EDGE_DIM]),
                        op=ALU.mult)
                    nc.vector.tensor_tensor(
                        out=mov[:, HEADS * EDGE_DIM + HEADS:].rearrange(
                            "p (h c) -> p h c", c=C),
                        in0=kvb[:, ti, HID:].rearrange("p (h c) -> p h c", c=C),
                        in1=al.rearrange("p (h o) -> p h o", o=1
                                         ).to_broadcast([P, HEADS, C]),
                        op=ALU.mult)
                    BD = HEADS * EDGE_DIM + HEADS  # 68
                    if ti < T - 1:
                        nc.tensor.matmul(ps_all[:], aeb[:, ti, 0:P], mov[:],
                                         start=(ti == 0), stop=False)
                    else:
                        nc.tensor.matmul(ps_all[:, :BD], aeb[:, ti, 0:P],
                                         mov[:, :BD], start=(ti == 0), stop=True)
                        nc.tensor.matmul(ps_all[:, BD:], aeb[:, ti, 0:P],
                                         mov[:, BD:], start=(ti == 0), stop=False)
                # ---- group tail: B @ We_blk, den copy ----
                BD = HEADS * EDGE_DIM + HEADS
                bsb = sb.tile([P, HEADS * EDGE_DIM], BF, tag="bsb", name="bsb")
                nc.scalar.activation(bsb[:], ps_all[:, :HEADS * EDGE_DIM], ACTF.Copy)
                nc.scalar.activation(den_sb[:, g * HEADS:(g + 1) * HEADS],
                                     ps_all[:, HEADS * EDGE_DIM:BD], ACTF.Copy)
                tp = psum.tile([HEADS * EDGE_DIM, P], BF, tag="tp", name="btp", bufs=1)
                nc.tensor.transpose(tp[:], bsb[:], ident[:])
                btsb = sb.tile([HEADS * EDGE_DIM, P], BF, tag="btsb", name="btsb")
                nc.scalar.activation(btsb[:], tp[:], ACTF.Copy)
                nc.tensor.matmul(ps_all[:, BD:], btsb[:], Web_sb[:],
                                 start=False, stop=True)

                # ---- fused node phase A: normalize, beta-gate, BN sums ----
                if SKIP_A:
                    nc.scalar.activation(xnew[:, g * HID:(g + 1) * HID],
                                         ps_all[:, BD:], ACTF.Copy)
                    toff += T
                    continue
                den = den_sb[:, g * HEADS:(g + 1) * HEADS]
                dmx = sb.tile([P, HEADS], F32, tag="dmx", name="dmx")
                nc.vector.tensor_scalar_max(dmx[:], den, 1e-30)
                rden = sb.tile([P, HEADS], F32, tag="rden", name="rden")
                nc.vector.reciprocal(rden[:], dmx[:])
                outn = sb.tile([P, HID], BF, tag="outn", name="outn")
                nc.vector.tensor_tensor(
                    out=outn[:].rearrange("p (h c) -> p h c", c=C),
                    in0=ps_all[:, BD:].rearrange("p (h c) -> p h c", c=C),
                    in1=rden[:].rearrange("p (h o) -> p h o", o=1
                                          ).to_broadcast([P, HEADS, C]),
                    op=ALU.mult)
                xps = proj_psum(g, 3)  # x_r = x@Wskip + bskip
                xr = sb.tile([P, HID], BF, tag="xr", name="xr")
                nc.scalar.activation(xr[:], xps[:, :HID], ACTF.Copy)
                bl = sb.tile([P, 2], F32, tag="bl", name="bl")
                scrb = sb.tile([P, HID], BF, tag="scrb", name="scrb")
                nc.vector.scalar_tensor_tensor(
                    out=scrb[:], in0=outn[:], scalar=1.0, in1=wa_sb[:],
                    op0=ALU.bypass, op1=ALU.mult, accum_out=bl[:, 0:1])
                nc.vector.scalar_tensor_tensor(
                    out=scrb[:], in0=xr[:], scalar=1.0, in1=wb_sb[:],
                    op0=ALU.bypass, op1=ALU.mult, accum_out=bl[:, 1:2])
                blsum = sb.tile([P, 1], F32, tag="blsum", name="blsum")
                nc.vector.tensor_add(blsum[:], bl[:, 0:1], bl[:, 1:2])
                # sigmoid via the Exp table (already loaded for alpha):
                # avoids 2 ACT table reloads per group
                bex = sb.tile([P, 1], F32, tag="bex", name="bex")
                nc.scalar.activation(bex[:], blsum[:], ACTF.Exp, scale=-1.0)
                nc.vector.tensor_scalar_add(bex[:], bex[:], 1.0)
                bsig = sb.tile([P, 1], F32, tag="bsig", name="bsig")
                nc.vector.reciprocal(bsig[:], bex[:])
                diff = sb.tile([P, HID], BF, tag="diff", name="diff")
                nc.vector.tensor_sub(diff[:], xr[:], outn[:])
                bd_t = sb.tile([P, HID], BF, tag="bd", name="bd")
                nc.vector.tensor_scalar_mul(bd_t[:], diff[:], bsig[:, 0:1])
                xn = xnew[:, g * HID:(g + 1) * HID]
                nc.vector.tensor_add(xn, outn[:], bd_t[:])
                nc.vector.tensor_scalar_mul(xn, xn, mct_sb[:, g:g + 1])
                x2 = sb.tile([P, HID], BF, tag="x2", name="x2")
                nc.vector.tensor_mul(x2[:], xn, xn)
                ci, c_first, c_last = ch_of[g]
                if c_first:
                    cur_ps[0] = psum.tile([1, 2 * HID], F32, tag="stats",
                                          name=f"stps{ci}", bufs=2)
                ps_st = cur_ps[0]
                nc.tensor.matmul(ps_st[:, :HID], ones_col[:], xn,
                                 start=c_first, stop=c_last)
                nc.tensor.matmul(ps_st[:, HID:], ones_col[:], x2[:],
                                 start=c_first, stop=c_last)
                if c_last:
                    nc.scalar.activation(
                        st_part[:, ci * 2 * HID:(ci + 1) * 2 * HID],
                        ps_st[:], ACTF.Copy)
                toff += T
            nc.leave_named_scope(f"edge{l}", _sc_edge, False)

            if KB_DUMP:
                for tt_ in range(NG):
                    dtmp2 = sb.tile([P, HID], F32, tag="dtmp", name="dtmp2")
                    nc.vector.tensor_copy(dtmp2[:], xnew[:, tt_ * HID:(tt_ + 1) * HID])
                    nc.sync.dma_start(
                        out=d_xn[:, (l * NG + tt_) * HID:(l * NG + tt_ + 1) * HID],
                        in_=dtmp2[:])
                if l == 0:
                    nc.sync.dma_start(out=d_den[:], in_=den_sb[:])

            # ---- BN stats all-reduce + finalize ----
            if "S" in KB_SKIP:
                continue
            _sc_bn = nc.enter_named_scope(f"bnst{l}", False)[0]
            st_fin = sb.tile([1, 2 * HID], F32, tag="stfin", name="stfin")
            if SKIP_A or not active:
                nc.vector.tensor_copy(st_fin[:], cur_ps[0][:])
            elif NCH == 1:
                nc.vector.tensor_copy(st_fin[:], st_part[:, 0:2 * HID])
            else:
                nc.vector.tensor_add(st_fin[:], st_part[:, 0:2 * HID],
                                     st_part[:, 2 * HID:4 * HID])
                for ci in range(2, NCH):
                    nc.vector.tensor_add(
                        st_fin[:], st_fin[:],
                        st_part[:, ci * 2 * HID:(ci + 1) * 2 * HID])
            st_a = sb.tile([1, HID], F32, tag="stsa", name="stsa")
            nc.vector.tensor_copy(st_a[:], st_fin[:, :HID])
            st_b = sb.tile([1, HID], F32, tag="stsb", name="stsb")
            nc.vector.tensor_copy(st_b[:], st_fin[:, HID:])
            nc.sync.dma_start(out=st_in[0:1, :], in_=st_a[:])
            nc.sync.dma_start(out=st_in[1:2, :], in_=st_b[:])
            if SIM1:
                nc.sync.dma_start(out=st_out[:], in_=st_in[:])
            else:
                nc.gpsimd.collective_compute("AllReduce", ALU.add,
                                             replica_groups=cg,
                                             ins=[st_in[:].opt()], outs=[st_out[:].opt()])
            str_a = sb.tile([1, HID], F32, tag="stra", name="stra")
            nc.sync.dma_start(out=str_a[:], in_=st_out[0:1, :])
            str_b = sb.tile([1, HID], F32, tag="strb", name="strb")
            nc.sync.dma_start(out=str_b[:], in_=st_out[1:2, :])
            mean = sb.tile([1, HID], F32, tag="mean", name="mean")
            nc.vector.tensor_scalar_mul(mean[:], str_a[:], 1.0 / N)
            var = sb.tile([1, HID], F32, tag="var", name="var")
            nc.vector.tensor_scalar_mul(var[:], str_b[:], 1.0 / N)
            msq = sb.tile([1, HID], F32, tag="msq", name="msq")
            nc.vector.tensor_mul(msq[:], mean[:], mean[:])
            nc.vector.tensor_sub(var[:], var[:], msq[:])
            nc.vector.tensor_scalar_add(var[:], var[:], EPS)
            sd = sb.tile([1, HID], F32, tag="sd", name="sd")
            nc.scalar.activation(sd[:], var[:], ACTF.Sqrt)
            rstd = sb.tile([1, HID], F32, tag="rstd", name="rstd")
            nc.vector.reciprocal(rstd[:], sd[:])
            # one Newton step for 1/sqrt(var): r <- r*(1.5 - 0.5*var*r^2)
            # (the ACT Sqrt table is only ~0.3% accurate; this error is a
            # per-channel systematic scale on x that compounds across layers)
            nt = sb.tile([1, HID], F32, tag="nt", name="nt")
            nc.vector.tensor_mul(nt[:], rstd[:], rstd[:])
            nc.vector.tensor_mul(nt[:], nt[:], var[:])
            nc.vector.tensor_scalar_mul(nt[:], nt[:], -0.5)
            nc.vector.tensor_scalar_add(nt[:], nt[:], 1.5)
            nc.vector.tensor_mul(rstd[:], rstd[:], nt[:])
            gam_sb = sb.tile([1, HID], F32, tag="gamsb", name="gamsb")
            nc.sync.dma_start(out=gam_sb[:], in_=gam[l][:])
            bet_sb = sb.tile([1, HID], F32, tag="betsb", name="betsb")
            nc.sync.dma_start(out=bet_sb[:], in_=bet[l][:])
            aa = sb.tile([1, HID], BF, tag="aa", name="aa")
            nc.vector.tensor_mul(aa[:], gam_sb[:], rstd[:])
            ac_ = sb.tile([1, HID], F32, tag="acs", name="acs")
            nc.vector.tensor_mul(ac_[:], mean[:], aa[:])
            ccs = sb.tile([1, HID], BF, tag="ccs", name="ccs")
            nc.vector.tensor_sub(ccs[:], bet_sb[:], ac_[:])
            for h in range(2):
                tpa = psum.tile([P, 2], F32, tag="tp", name="tpa", bufs=1)
                nc.tensor.matmul(tpa[:, 0:1], aa[0:1, h * P:(h + 1) * P], one1[:],
                                 start=True, stop=True)
                nc.tensor.matmul(tpa[:, 1:2], ccs[0:1, h * P:(h + 1) * P], one1[:],
                                 start=True, stop=True)
                nc.vector.tensor_copy(aT_sb[:, h:h + 1], tpa[:, 0:1])
                nc.vector.tensor_copy(cT_sb[:, h:h + 1], tpa[:, 1:2])
            nc.leave_named_scope(f"bnst{l}", _sc_bn, False)

            # ---- node phase B: transpose + BN apply + LeakyReLU -> x_T ----
            if l < L - 1 and "B" in KB_SKIP:
                load_layer_weights(l + 1)
                qkv_phase()
            elif l < L - 1:
                _sc_nB = nc.enter_named_scope(f"nodeB{l}", False)[0]
                mrep = sb.tile([P, P], BF, tag="mrep", name="mrep")
                nc.sync.dma_start(out=mrep[:], in_=maskrep[:])
                for t in range(NG):
                    for h in range(2):
                        tp = psum.tile([P, P], BF, tag="tp", name="xtp", bufs=1)
                        nc.tensor.transpose(
                            tp[:], xnew[:, t * HID + h * P:t * HID + (h + 1) * P],
                            ident[:])
                        ybn = sb.tile([P, P], BF, tag="ybn2", name="ybn2")
                        nc.scalar.activation(ybn[:], tp[:],
                                             ACTF.Identity,
                                             bias=cT_sb[:, h:h + 1],
                                             scale=aT_sb[:, h:h + 1])
                        y1 = sb.tile([P, P], BF, tag="y1b", name="y1b")
                        nc.vector.tensor_scalar_mul(y1[:], ybn[:], 0.1)
                        dst = x_T[h][:, t * P:(t + 1) * P]
                        nc.vector.tensor_max(dst, ybn[:], y1[:])
                        if t == NG - 1:
                            nc.vector.tensor_mul(dst, dst, mrep[:])
                nc.leave_named_scope(f"nodeB{l}", _sc_nB, False)
                if KB_DUMP and l == 0:
                    for h in range(2):
                        for tt_ in range(NG):
                            dxt = sb.tile([P, P], F32, tag="dxt", name="dxt", bufs=2)
                            nc.vector.tensor_copy(dxt[:], x_T[h][:, tt_ * P:(tt_ + 1) * P])
                            nc.sync.dma_start(
                                out=d_xt[:, h * n_pad + tt_ * P:h * n_pad + (tt_ + 1) * P],
                                in_=dxt[:])
                    dbn = sb.tile([1, 2 * HID], F32, tag="dbn", name="dbn")
                    nc.vector.tensor_copy(dbn[:, :HID], aa[:])
                    nc.vector.tensor_copy(dbn[:, HID:], ccs[:])
                    nc.sync.dma_start(out=d_bn[:], in_=dbn[:])
                load_layer_weights(l + 1)
                qkv_phase()
            else:
                # ---- pooling + head (x of last layer = BN+lrelu of xnew) ----
                _sc_pool = nc.enter_named_scope("pool", False)[0]
                arep_ps = psum.tile([P, QW], F32, tag="mm", name="arep")
                nc.tensor.matmul(arep_ps[:, :HID], ones_row[:], aa[:], start=True, stop=True)
                arep = sb.tile([P, HID], BF, tag="arep", name="arepsb")
                nc.vector.tensor_copy(arep[:], arep_ps[:, :HID])
                crep_ps = psum.tile([P, QW], F32, tag="mm", name="crep")
                nc.tensor.matmul(crep_ps[:, :HID], ones_row[:], ccs[:], start=True, stop=True)
                crep = sb.tile([P, HID], BF, tag="crep", name="crepsb")
                nc.vector.tensor_copy(crep[:], crep_ps[:, :HID])
                ps_pool = psum.tile([G, 1], F32, tag="stats", name="poolps", bufs=2)
                if "pool" in KB_SKIP:
                    ptd = sb.tile([P, G], BF, tag="ptsb", name="ptd")
                    nc.vector.memset(ptd[:], 0.0)
                    nc.tensor.matmul(ps_pool[:], ptd[:], ones_col[:, 0:1],
                                     start=True, stop=True)
                for t in range(NG if "pool" not in KB_SKIP else 0):
                    xn = xnew[:, t * HID:(t + 1) * HID]
                    y1 = sb.tile([P, HID], BF, tag="y1", name="y1")
                    nc.vector.tensor_mul(y1[:], xn, arep[:])
                    ybn = sb.tile([P, HID], BF, tag="ybn", name="ybn")
                    nc.vector.tensor_add(ybn[:], y1[:], crep[:])
                    yr = sb.tile([P, HID], BF, tag="yr", name="yr")
                    # leaky relu: max(x, 0.1x)
                    nc.vector.tensor_scalar_mul(y1[:], ybn[:], 0.1)
                    nc.vector.tensor_max(yr[:], ybn[:], y1[:])
                    s1 = sb.tile([P, 2], F32, tag="s1", name="s1")
                    scrb2 = sb.tile([P, HID], BF, tag="scrb", name="scrb2")
                    nc.vector.scalar_tensor_tensor(
                        out=scrb2[:], in0=yr[:], scalar=1.0, in1=h1r_sb[:],
                        op0=ALU.bypass, op1=ALU.mult, accum_out=s1[:, 0:1])
                    nc.vector.scalar_tensor_tensor(
                        out=scrb2[:], in0=yr[:], scalar=1.0, in1=h2r_sb[:],
                        op0=ALU.bypass, op1=ALU.mult, accum_out=s1[:, 1:2])
                    yv = sb.tile([P, 1], F32, tag="yv", name="yv")
                    nc.vector.tensor_mul(yv[:], s1[:, 0:1], ict_sb[:, t:t + 1])
                    yw = sb.tile([P, 1], BF, tag="yw", name="yw")
                    nc.vector.tensor_add(yw[:], yv[:], s1[:, 1:2])
                    # mask pads (bn shifts pads off zero)
                    nc.vector.tensor_scalar_mul(yw[:], yw[:], mct_sb[:, t:t + 1])
                    pt_sb = sb.tile([P, G], BF, tag="ptsb", name="ptsb")
                    nc.sync.dma_start(out=pt_sb[:], in_=p_t[t * P:(t + 1) * P, :])
                    nc.tensor.matmul(ps_pool[:], pt_sb[:], yw[:],
                                     start=(t == 0), stop=(t == NG - 1))
                pool_sb = sb.tile([G, 1], F32, tag="poolsb", name="poolsb")
                nc.vector.tensor_copy(pool_sb[:], ps_pool[:])
                nc.sync.dma_start(out=pool_in[:], in_=pool_sb[:])
                if SIM1:
                    nc.sync.dma_start(out=pool_out[:], in_=pool_in[:])
                else:
                    nc.gpsimd.collective_compute("AllReduce", ALU.add,
                                                 replica_groups=cg,
                                                 ins=[pool_in[:].opt()],
                                                 outs=[pool_out[:].opt()])
                pr = sb.tile([G, 1], F32, tag="pr", name="pr")
                nc.sync.dma_start(out=pr[:], in_=pool_out[:])
                fin = sb.tile([G, 1], F32, tag="fin", name="fin")
                nc.vector.tensor_scalar_add(fin[:], pr[:], float(head_b))
                nc.sync.dma_start(out=out_ext[:], in_=fin[:])
                nc.leave_named_scope("pool", _sc_pool, False)

        ctx.close()
    return nc


LAST_RESULT = None


def kernel(**inputs):
    global LAST_RESULT
    meta, cores = plan(inputs["edge_index"], inputs["batch"])
    in_maps = build_inmaps(inputs, meta, cores)
    head_b = float(np.asarray(inputs["head_b"]).reshape(-1)[0])
    nc = bacc.Bacc("TRN2")
    build(nc, meta, head_b)
    if not nc.is_finalized():
        nc.finalize()
    res = run_bass_kernel_spmd(nc, in_maps, core_ids=list(range(NCORES)))
    LAST_RESULT = res
    out = np.asarray(res.results[0]["out"], dtype=np.float32).reshape(G)
    return out


if __name__ == "__main__":
    import reference
    inputs = {k: np.asarray(v) for k, v in reference.setup_inputs().items()}
    got = kernel(**inputs)
    exp = np.asarray(reference.reference(**inputs))
    rel = np.abs(got - exp).max() / (np.abs(exp).max() + 1e-9)
    print("Relative error:", rel)
